# revision 1
# baseline (speedup 1.0000x reference)
"""ArcticDecoderLayer on 8 TRN2 NeuronCores.

Sharding strategy (expert-parallel per the hint):
 - MoE: core c owns expert c (e_w1/e_w3/e_w2 sharded on the expert axis).
   Each core computes silu(x @ w1_c) * (x @ w3_c), scales rows by that
   expert's normalized top-2 routing weight (0 for tokens not routed to
   it -- masked-dense, mathematically exact), then @ w2_c. Host sums the
   8 partial outputs (the gather/unshard step).
 - Dense residual MLP: column-sharded across cores (core c gets 256 of
   the 2048 ffn columns of res_w1/res_w3 and the matching 256 rows of
   res_w2); partials summed on the host with the MoE partials.
 - Attention / norms / gate are tiny (<12% of layer FLOPs) and run on
   the host as part of input prep; routing weights feed the device as
   per-core row-scale vectors.

Device matmuls run in bf16 (fp32 PSUM accumulation) via matmul_tile_kernel.
"""

import numpy as np

from concourse import bacc, mybir, tile
import concourse.bass as bass
from concourse.bass_utils import run_bass_kernel_spmd
from concourse.kernels.tile_matmul import matmul_tile_kernel

B, S, H = 1, 1024, 2048
NH, HD, KVH = 32, 64, 8
E, F, TOPK = 8, 2048, 2
EPS = 1e-6
ROPE_THETA = 10000.0
N_CORES = 8
FSH = F // N_CORES  # res-mlp ffn shard = 256

LAST_RESULTS = None  # stashed BassKernelResults for test harnesses

_COMPILED = {}


def _build_nc():
    nc = bacc.Bacc("TRN2", target_bir_lowering=False, debug=False,
                   num_devices=N_CORES)
    f32 = mybir.dt.float32
    bf16 = mybir.dt.bfloat16

    xT = nc.dram_tensor("xT", [H, S], f32, kind="ExternalInput")
    hrT = nc.dram_tensor("hrT", [H, S], f32, kind="ExternalInput")
    ew1 = nc.dram_tensor("ew1", [H, F], f32, kind="ExternalInput")
    ew3 = nc.dram_tensor("ew3", [H, F], f32, kind="ExternalInput")
    ew2 = nc.dram_tensor("ew2", [F, H], f32, kind="ExternalInput")
    rw1 = nc.dram_tensor("rw1", [H, FSH], f32, kind="ExternalInput")
    rw3 = nc.dram_tensor("rw3", [H, FSH], f32, kind="ExternalInput")
    rw2 = nc.dram_tensor("rw2", [FSH, H], f32, kind="ExternalInput")
    wvec = nc.dram_tensor("wvec", [1, S], f32, kind="ExternalInput")
    moe_out = nc.dram_tensor("moe_out", [S, H], f32, kind="ExternalOutput")
    res_out = nc.dram_tensor("res_out", [S, H], f32, kind="ExternalOutput")

    with tile.TileContext(nc) as tc:
        with tc.tile_pool(name="dram", bufs=1, space="DRAM") as dram:
            up1T = dram.tile([F, S], f32, tag="up1T")
            up3T = dram.tile([F, S], f32, tag="up3T")
            hT = dram.tile([F, S], bf16, tag="hT")
            up1rT = dram.tile([FSH, S], f32, tag="up1rT")
            up3rT = dram.tile([FSH, S], f32, tag="up3rT")
            hrsT = dram.tile([FSH, S], bf16, tag="hrsT")

            # (x @ w)^T = w^T @ x -> kxm = w [H, F], kxn = xT [H, S]
            matmul_tile_kernel(tc, ew1[:], xT[:], up1T[:],
                               matmul_dtype=bf16)
            matmul_tile_kernel(tc, ew3[:], xT[:], up3T[:],
                               matmul_dtype=bf16)
            matmul_tile_kernel(tc, rw1[:], hrT[:], up1rT[:],
                               matmul_dtype=bf16)
            matmul_tile_kernel(tc, rw3[:], hrT[:], up3rT[:],
                               matmul_dtype=bf16)

            with tc.tile_pool(name="ew", bufs=3) as pool, \
                 tc.tile_pool(name="wrow", bufs=1) as wpool:
                # routing weights broadcast to all 128 partitions
                wv_ap = wvec[:]
                wb = wpool.tile([128, S], mybir.dt.float32, tag="wb")
                bcast = bass.AP(tensor=wv_ap.tensor, offset=wv_ap.offset,
                                ap=[[0, 128], wv_ap.ap[-1]])
                nc.gpsimd.dma_start(out=wb[:], in_=bcast)

                def gated(srcs1, srcs3, dst, ntiles, scale):
                    for i in range(ntiles):
                        sl = slice(i * 128, (i + 1) * 128)
                        t1 = pool.tile([128, S], mybir.dt.float32, tag="t1")
                        t3 = pool.tile([128, S], mybir.dt.float32, tag="t3")
                        nc.sync.dma_start(out=t1[:], in_=srcs1[sl, :])
                        nc.sync.dma_start(out=t3[:], in_=srcs3[sl, :])
                        sm = pool.tile([128, S], mybir.dt.float32, tag="sm")
                        nc.scalar.activation(
                            sm[:], t1[:], mybir.ActivationFunctionType.Silu)
                        nc.vector.tensor_mul(sm[:], sm[:], t3[:])
                        if scale:
                            nc.vector.tensor_mul(sm[:], sm[:], wb[:])
                        hb = pool.tile([128, S], mybir.dt.bfloat16, tag="hb")
                        nc.vector.tensor_copy(hb[:], sm[:])
                        nc.sync.dma_start(out=dst[sl, :], in_=hb[:])

                gated(up1T, up3T, hT, F // 128, True)
                gated(up1rT, up3rT, hrsT, FSH // 128, False)

            # moe = (hT)^T @ ew2 ; res = (hrsT)^T @ rw2
            matmul_tile_kernel(tc, hT[:], ew2[:], moe_out[:],
                               matmul_dtype=bf16)
            matmul_tile_kernel(tc, hrsT[:], rw2[:], res_out[:],
                               matmul_dtype=bf16)

    nc.compile()
    return nc


def _np_softmax(x, axis=-1):
    m = np.max(x, axis=axis, keepdims=True)
    e = np.exp(x - m)
    return e / np.sum(e, axis=axis, keepdims=True)


def _rmsnorm(x, w):
    v = np.mean(np.square(x), axis=-1, keepdims=True)
    return x / np.sqrt(v + EPS) * w


def kernel(hidden_states, attention_mask, position_ids, wq, wk, wv, wo,
           norm1_w, norm_res_w, res_w1, res_w3, res_w2,
           gate_w, e_w1, e_w3, e_w2):
    global LAST_RESULTS
    f4 = np.float32
    x = np.asarray(hidden_states, f4).reshape(S, H)
    amask = np.asarray(attention_mask).reshape(S)
    pos = np.asarray(position_ids).reshape(S).astype(np.int64)

    # ---- host: attention (tiny vs MoE) ----
    inv_freq = 1.0 / (ROPE_THETA ** (np.arange(0, HD, 2, dtype=f4) / HD))
    t = np.arange(S, dtype=f4)
    freqs = np.outer(t, inv_freq)
    emb = np.concatenate([freqs, freqs], axis=-1)
    sin_t, cos_t = np.sin(emb), np.cos(emb)
    s_ = sin_t[pos].astype(f4)
    c_ = cos_t[pos].astype(f4)

    h = _rmsnorm(x, np.asarray(norm1_w, f4))
    q = (h @ np.asarray(wq, f4)).reshape(S, NH, HD).transpose(1, 0, 2)
    k = (h @ np.asarray(wk, f4)).reshape(S, KVH, HD).transpose(1, 0, 2)
    v = (h @ np.asarray(wv, f4)).reshape(S, KVH, HD).transpose(1, 0, 2)

    def rot(z):
        hh = z.shape[-1] // 2
        return np.concatenate([-z[..., hh:], z[..., :hh]], axis=-1)

    q = q * c_[None] + rot(q) * s_[None]
    k = k * c_[None] + rot(k) * s_[None]
    groups = NH // KVH
    k = np.repeat(k, groups, axis=0)
    v = np.repeat(v, groups, axis=0)
    causal = np.tril(np.ones((S, S), bool))
    mask = causal & (amask > 0)[None, :]
    bias = np.where(mask, f4(0.0), np.finfo(f4).min).astype(f4)
    scores = np.einsum('hqd,hkd->hqk', q, k).astype(f4) * f4(1.0 / np.sqrt(HD))
    scores = scores + bias[None]
    p = _np_softmax(scores, axis=-1).astype(f4)
    attn = np.einsum('hqk,hkd->hqd', p, v).transpose(1, 0, 2).reshape(S, H)
    attn = attn @ np.asarray(wo, f4)
    h1 = x + attn
    hr = _rmsnorm(h1, np.asarray(norm_res_w, f4))

    # ---- host: top-2 routing -> per-expert row-scale vectors ----
    logits = x @ np.asarray(gate_w, f4)
    rw_ = _np_softmax(logits.astype(f4), axis=-1)
    ti = np.argsort(-rw_, axis=-1, kind="stable")[:, :TOPK]
    tw = np.take_along_axis(rw_, ti, axis=-1)
    tw = tw / np.sum(tw, axis=-1, keepdims=True)
    wdense = np.zeros((S, E), f4)
    np.add.at(wdense, (np.arange(S)[:, None], ti), tw)

    # ---- device: expert-parallel MoE + column-sharded residual MLP ----
    if "nc" not in _COMPILED:
        _COMPILED["nc"] = _build_nc()
    nc = _COMPILED["nc"]

    xT = np.ascontiguousarray(x.T)
    hrT = np.ascontiguousarray(hr.astype(f4).T)
    e_w1 = np.asarray(e_w1, f4)
    e_w3 = np.asarray(e_w3, f4)
    e_w2 = np.asarray(e_w2, f4)
    res_w1 = np.asarray(res_w1, f4)
    res_w3 = np.asarray(res_w3, f4)
    res_w2 = np.asarray(res_w2, f4)

    in_maps = []
    for c in range(N_CORES):
        cs = slice(c * FSH, (c + 1) * FSH)
        in_maps.append({
            "xT": xT,
            "hrT": hrT,
            "ew1": np.ascontiguousarray(e_w1[c]),
            "ew3": np.ascontiguousarray(e_w3[c]),
            "ew2": np.ascontiguousarray(e_w2[c]),
            "rw1": np.ascontiguousarray(res_w1[:, cs]),
            "rw3": np.ascontiguousarray(res_w3[:, cs]),
            "rw2": np.ascontiguousarray(res_w2[cs, :]),
            "wvec": np.ascontiguousarray(wdense[:, c].reshape(1, S)),
        })

    res = run_bass_kernel_spmd(nc, in_maps, core_ids=list(range(N_CORES)))
    LAST_RESULTS = res

    out = h1.copy()
    for c in range(N_CORES):
        out += np.asarray(res.results[c]["moe_out"], f4)
        out += np.asarray(res.results[c]["res_out"], f4)
    return out.reshape(B, S, H).astype(np.float32)
